# revision 75
# baseline (speedup 1.0000x reference)
"""MHSA + BatchNorm + residual for Trainium2, SPMD across 8 NeuronCores.

Problem (hardcoded): x [B=2, C=1024, T=2048] fp32
  q/k/v = W @ x[b] + b  (1x1 conv, per batch)
  16 heads x 64 dims, softmax attention over T
  y = Wo @ out + bo ; BatchNorm1d over (B, T); return x + gamma*norm(y)+beta

Sharding v2 (head-parallel, zero redundant compute):
  core c = (batch b = c//4, head-group g = c%4). Each core projects
  Q/K/V only for its 4 heads (256 channels) over the FULL T, runs
  attention for those heads, then one 8-way AllToAll reshards the
  attention output so core j holds ALL 1024 channels for column slice
  [256j:256j+256) of both batches fused as a 512-wide free dim.
  Wo + BatchNorm run on that slice; stats AllGather as before.

  bv is folded into bo on the host (softmax rows sum to 1, so
  attn@(v+bv) = attn@v + bv, and Wo@(out+bv)+bo = Wo@out + (Wo@bv+bo)).

dtypes: bf16 for all matmul operands (fp32 PSUM accumulate); fp32 for
biases/stats/residual/output.
"""

import numpy as np
import ml_dtypes

import concourse.bass as bass
import concourse.mybir as mybir
import concourse.tile as tile
from concourse import bacc
from concourse.bass_utils import run_bass_kernel_spmd

# problem dims
B, C, T, H, DH = 2, 1024, 2048, 16, 64
P = 128
KO = C // P            # 8 input-channel tiles
LO = 2                 # local output-channel tiles (256 chans / 128)
NT = T // P            # 16 s-tiles
NW = 4                 # q-chunks of 512
W5 = 512               # free dim per matmul
SCALE = DH ** -0.5     # 0.125
EPS = 1e-5
NCORES = 8
NBT = B * T            # BatchNorm count
CS = 256               # output column slice per core (per batch)

F32 = mybir.dt.float32
F32R = mybir.dt.float32r
BF16 = mybir.dt.bfloat16

TRACE = False          # test.py flips this for profiling
DEBUG = False          # adds intermediate-tensor outputs
LAST_RESULT = None     # BassKernelResults of the last run

_cached_nc = None


def _build():
    nc = bacc.Bacc("TRN2", target_bir_lowering=False, debug=False,
                   num_devices=NCORES)

    # all big inputs are pre-arranged partition-major on the host so each
    # DMA line is one long contiguous chunk per partition
    xkv_d = nc.dram_tensor("xkv", [NW, P, KO, W5], BF16,
                           kind="ExternalInput").ap()
    wqT_d = nc.dram_tensor("wqT", [P, KO, 2 * P], BF16,
                           kind="ExternalInput").ap()
    wkT_d = nc.dram_tensor("wkT", [P, KO, 2 * P], BF16,
                           kind="ExternalInput").ap()
    wvT_d = nc.dram_tensor("wvT", [P, KO, 2 * P], BF16,
                           kind="ExternalInput").ap()
    woT_d = nc.dram_tensor("woT", [P, KO, C], BF16,
                           kind="ExternalInput").ap()
    bq_d = nc.dram_tensor("bq", [2 * P], F32, kind="ExternalInput").ap()
    bk_d = nc.dram_tensor("bk", [2 * P], F32, kind="ExternalInput").ap()
    bo_d = nc.dram_tensor("bo", [C], F32, kind="ExternalInput").ap()
    gamma_d = nc.dram_tensor("gamma", [C], F32, kind="ExternalInput").ap()
    beta_d = nc.dram_tensor("beta", [C], F32, kind="ExternalInput").ap()
    xres_d = nc.dram_tensor("xres", [P, KO, W5], F32,
                            kind="ExternalInput").ap()
    sel_d = nc.dram_tensor("sel", [P, P], BF16, kind="ExternalInput").ap()
    if DEBUG:
        dbg_oo_d = nc.dram_tensor("dbg_oo", [P, LO, T], BF16,
                                  kind="ExternalOutput").ap()
        dbg_den_d = nc.dram_tensor("dbg_den", [P, LO, T], BF16,
                                   kind="ExternalOutput").ap()
        dbg_rhs_d = nc.dram_tensor("dbg_rhs", [P, KO, W5], BF16,
                                   kind="ExternalOutput").ap()
        dbg_k_d = nc.dram_tensor("dbg_k", [P, LO, T], BF16,
                                 kind="ExternalOutput").ap()
        dbg_q_d = nc.dram_tensor("dbg_q", [P, LO, T], BF16,
                                 kind="ExternalOutput").ap()
        dbg_vp_d = nc.dram_tensor("dbg_vp", [P, NT, 4, DH + 1], BF16,
                                  kind="ExternalOutput").ap()
    out_d = nc.dram_tensor("out", [P, KO, W5], F32,
                           kind="ExternalOutput").ap()

    # [C] -> [P, C//P] so channel c sits at (partition c%128, free c//128)
    def chan_vec(ap, o=KO):
        return ap.rearrange("(o p) -> p o", p=P)

    def chan_mat(ap):
        return ap.rearrange("(o p) n -> p o n", p=P)

    with tile.TileContext(nc) as tc:
        with (
            tc.tile_pool(name="consts", bufs=1) as consts,
            tc.tile_pool(name="persist", bufs=1) as persist,
            tc.tile_pool(name="dram", bufs=1, space="DRAM") as drampool,
        ):
            # warmup collective: inits/syncs RDH channels while inputs load
            wu_sb = consts.tile([1, 16], F32, name="wu_sb")
            nc.vector.memset(wu_sb[:], 1.0)
            wu_in = drampool.tile([1, 16], F32, name="wu_in")
            wu_out = drampool.tile([1, 16], F32, name="wu_out")
            nc.sync.dma_start(wu_in[:], wu_sb[:])
            nc.gpsimd.collective_compute(
                "AllReduce",
                mybir.AluOpType.add,
                replica_groups=[list(range(NCORES))],
                ins=[wu_in[:].opt()],
                outs=[wu_out[:].opt()],
            )

            # ---- constants ----
            bq_sb = consts.tile([P, LO], F32, name="bq_sb")
            nc.sync.dma_start(bq_sb[:], chan_vec(bq_d, LO))
            bk_sb = consts.tile([P, LO], F32, name="bk_sb")
            nc.sync.dma_start(bk_sb[:], chan_vec(bk_d, LO))
            bo_sb = consts.tile([P, KO], F32, name="bo_sb")
            nc.sync.dma_start(bo_sb[:], chan_vec(bo_d))
            gamma_sb = consts.tile([P, KO], F32, name="gamma_sb")
            nc.sync.dma_start(gamma_sb[:], chan_vec(gamma_d))
            beta_sb = consts.tile([P, KO], F32, name="beta_sb")
            nc.sync.dma_start(beta_sb[:], chan_vec(beta_d))
            eps_sb = consts.tile([P, 1], F32, name="eps_sb")
            nc.gpsimd.memset(eps_sb[:], EPS)
            # touch Exp now so the ACT table load happens during input DMA;
            # input is a DMA-loaded const so nothing queues ahead of it
            warm_sb = consts.tile([P, 1], F32, name="warm_sb")
            nc.scalar.activation(warm_sb[:], bq_sb[:, 0:1],
                                 mybir.ActivationFunctionType.Exp)
            # sel[r, d] = 1 iff r == 64*(d//64): broadcasts den rows 0/64
            sel_sb = consts.tile([P, P], BF16, name="sel_sb")
            nc.sync.dma_start(sel_sb[:], sel_d)

            # ---- persistent activations ----
            k_sb = persist.tile([P, LO, T], BF16, name="k_sb")
            q_sb = persist.tile([P, LO, T], BF16, name="q_sb")
            # V' [t-part, s-tile, local head, DH+1]; col DH is the ones col
            vp_sb = persist.tile([P, NT, 4, DH + 1], BF16, name="vp_sb")
            nc.gpsimd.memset(vp_sb[:, :, :, DH:DH + 1], 1.0)
            out_own = persist.tile([P, LO, T], BF16, name="out_own")
            # den for head-in-pair e sits at partition 64e; other rows 1.0
            den_sb = persist.tile([P, LO, T], BF16, name="den_sb")
            nc.gpsimd.memset(den_sb[:], 1.0)
            s1_sb = persist.tile([P, KO], F32, name="s1_sb")
            s2_sb = persist.tile([P, KO], F32, name="s2_sb")
            wo_sb = persist.tile([P, KO, C], BF16, name="wo_sb")
            wo_rhs = persist.tile([P, KO, W5], BF16, name="wo_rhs")

            # per-pair AllToAll buffers (pair p <-> local o-tile p)
            a2a_in = [drampool.tile([NCORES, P, CS], BF16, name=f"a2a_in{p}")
                      for p in range(2)]
            a2a_out = [drampool.tile([NCORES, P, CS], BF16,
                                     name=f"a2a_out{p}") for p in range(2)]

            def emit_a2a(p):
                nc.gpsimd.collective_compute(
                    "AllToAll",
                    mybir.AluOpType.bypass,
                    replica_groups=[list(range(NCORES))],
                    ins=[a2a_in[p][:].opt()],
                    outs=[a2a_out[p][:].opt()],
                )
                # a2a_out[r=(b,g), p, c] -> wo_rhs[p, ki=2g+p, b*CS+c]
                re = a2a_out[p][:].rearrange("(b g) p c -> p g b c", b=B)
                for g in range(4):
                    ki = 2 * g + p
                    nc.sync.dma_start(
                        wo_rhs[:, ki, :].rearrange("p (b c) -> p b c", b=B),
                        re[:, g, :, :])

            with (
                tc.tile_pool(name="wpool", bufs=1) as wpool,
                tc.tile_pool(name="xpool", bufs=1) as xpool,
            ):
                # ---- weight + x loads (first-needed first) ----
                # per-ki split so the first K chain starts on 1/8 of the data
                wk_sb = wpool.tile([P, KO, 2 * P], BF16, name="wk_sb")
                # quarter-major so each per-quarter DMA lands contiguously
                xfull = xpool.tile([P, NW, KO, W5], BF16, name="xfull")
                for ki in range(KO):
                    nc.sync.dma_start(
                        wk_sb[:, ki, 0:P], wkT_d[:, ki, 0:P])
                    nc.sync.dma_start(
                        xfull[:, 0, ki, :], xkv_d[0][:, ki, :])
                nc.sync.dma_start(wk_sb[:, :, P:2 * P], wkT_d[:, :, P:2 * P])
                wq_sb = wpool.tile([P, KO, 2 * P], BF16, name="wq_sb")
                nc.sync.dma_start(wq_sb[:], wqT_d)
                for q in range(1, NW):
                    nc.sync.dma_start(xfull[:, q], xkv_d[q])
                wv_sb = wpool.tile([P, KO, 2 * P], BF16, name="wv_sb")
                nc.sync.dma_start(wv_sb[:], wvT_d)
                nc.sync.dma_start(wo_sb[:], woT_d)

                def proj_tile(pool, dst, w_sb, b_sb, o, q):
                    ps = pool.tile([P, W5], F32, name="proj_ps", tag="av")
                    for ki in range(KO):
                        nc.tensor.matmul(
                            ps[:],
                            w_sb[:, ki, o * P:(o + 1) * P],
                            xfull[:, q, ki, :],
                            start=(ki == 0), stop=(ki == KO - 1),
                        )
                    # drain on DVE, keeping ACT free for the exp stream
                    nc.vector.tensor_scalar_add(
                        dst[:, o, q * W5:(q + 1) * W5], ps[:],
                        b_sb[:, o:o + 1])

                # K for full T first (sims need all of K), then Q chunk 0
                with tc.tile_pool(name="kpsum", bufs=2,
                                  space="PSUM") as kpsum:
                    for q in range(NW):
                        for o in range(LO):
                            proj_tile(kpsum, k_sb, wk_sb, bk_sb, o, q)
                    for o in range(LO):
                        proj_tile(kpsum, q_sb, wq_sb, bq_sb, o, 0)

                # ---- attention: 2 head pairs x 4 q-chunks, pipelined ----
                with (
                    tc.tile_pool(name="spsum", bufs=2,
                                 space="PSUM") as spsum,
                    tc.tile_pool(name="apsum", bufs=4,
                                 space="PSUM") as apsum,
                    tc.tile_pool(name="epool", bufs=2) as epool,
                ):
                    def vproj_quarter(q):
                        for tt in range(4):
                            st = q * 4 + tt
                            ps = apsum.tile([P, 2 * P], F32, name="v_ps",
                                            tag="av")
                            for ki in range(KO):
                                nc.tensor.matmul(
                                    ps[:],
                                    xfull[:, q, ki, tt * P:(tt + 1) * P],
                                    wv_sb[:, ki, :],
                                    start=(ki == 0), stop=(ki == KO - 1),
                                )
                            nc.vector.tensor_copy(
                                vp_sb[:, st, :, 0:DH],
                                ps[:].rearrange("p (l d) -> p l d", d=DH))

                    def emit_sims(p, w):
                        # e2[:, st, e, :] = exp(scale * k_e^T q) for pair p
                        e2 = epool.tile([P, NT, 2, W5], BF16, name="e_sb",
                                        tag="e")
                        for s in range(NT):
                            pst = spsum.tile([P, 2, W5], F32,
                                             name="sim_ps", tag="sim")
                            for e in range(2):
                                pb = DH * e
                                nc.tensor.matmul(
                                    pst[:, e, :],
                                    k_sb[pb:pb + DH, p,
                                         s * P:(s + 1) * P],
                                    q_sb[pb:pb + DH, p,
                                         w * W5:(w + 1) * W5],
                                    start=True, stop=True,
                                )
                            nc.scalar.activation(
                                e2[:, s, :, :], pst[:],
                                mybir.ActivationFunctionType.Exp,
                                scale=SCALE)
                        return e2

                    def emit_av_div(p, w, e2):
                        wsl = slice(w * W5, (w + 1) * W5)
                        for e in range(2):
                            l = 2 * p + e
                            av = apsum.tile([DH + 1, W5], F32, name="av_ps",
                                            tag="av")
                            for st in range(NT):
                                nc.tensor.matmul(
                                    av[:],
                                    vp_sb[:, st, l, :],
                                    e2[:, st, e, :],
                                    start=(st == 0), stop=(st == NT - 1),
                                )
                            nc.vector.tensor_copy(
                                out_own[DH * e:DH * (e + 1), p, wsl],
                                av[0:DH, :])
                            nc.vector.tensor_copy(
                                den_sb[DH * e:DH * e + 1, p, wsl],
                                av[DH:DH + 1, :])
                        bc = apsum.tile([P, W5], F32, name="bc_ps", tag="av")
                        nc.tensor.matmul(
                            bc[:], sel_sb[:], den_sb[:, p, wsl],
                            start=True, stop=True)
                        bcr = epool.tile([P, W5], F32, name="bcr",
                                         tag="bcr")
                        nc.vector.reciprocal_approx_fast(bcr[:], bc[:])
                        nc.vector.tensor_tensor(
                            out_own[:, p, wsl], out_own[:, p, wsl], bcr[:],
                            mybir.AluOpType.mult)
                        # stage this (pair, chunk) into the AllToAll input
                        nc.sync.dma_start(
                            a2a_in[p][2 * w:2 * w + 2, :, :]
                            .rearrange("j p c -> p j c"),
                            out_own[:, p, wsl].rearrange(
                                "p (j c) -> p j c", j=2))

                    prev = None
                    qw = 1  # Q chunks emitted so far
                    for p in range(2):
                        for w in range(NW):
                            et = emit_sims(p, w)
                            if p == 0 and w == 0:
                                # all of V' must precede the first AV emission
                                for vq in range(NW):
                                    vproj_quarter(vq)
                            if qw < NW:
                                for o in range(LO):
                                    proj_tile(apsum, q_sb, wq_sb, bq_sb,
                                              o, qw)
                                qw += 1
                            if prev is not None:
                                emit_av_div(*prev)
                                if prev[0] == 0 and prev[1] == NW - 1:
                                    emit_a2a(0)  # pair-0 exchange, hidden
                            prev = (p, w, et)
                    emit_av_div(*prev)

            if DEBUG:
                nc.sync.dma_start(dbg_oo_d, out_own[:])
                nc.sync.dma_start(dbg_den_d, den_sb[:])
                nc.sync.dma_start(dbg_k_d, k_sb[:])
                nc.sync.dma_start(dbg_q_d, q_sb[:])
                nc.sync.dma_start(dbg_vp_d, vp_sb[:])

            emit_a2a(1)

            # ---- Wo + BN stats on own column slice ----
            with (
                tc.tile_pool(name="post", bufs=1) as post,
                tc.tile_pool(name="ypsum", bufs=1, space="PSUM") as ypsum,
                tc.tile_pool(name="hpool", bufs=2) as hpool,
                tc.tile_pool(name="scratch", bufs=2) as scratch,
            ):
                if DEBUG:
                    nc.sync.dma_start(dbg_rhs_d, wo_rhs[:])
                y_sb = post.tile([P, KO, W5], F32, name="y_sb")
                xres_sb = post.tile([P, KO, W5], F32, name="xres_sb")
                nc.sync.dma_start(xres_sb[:], xres_d)

                # even ki (from the pair-0 exchange) first: those matmuls
                # run while the pair-1 AllToAll is still in flight; the odd
                # phase goes m-by-m so drains/stats pipeline with the MMs
                yps = [ypsum.tile([P, W5], F32, name="y_ps", tag=f"yp{m}")
                       for m in range(KO)]
                for idx, ki in enumerate((0, 2, 4, 6)):
                    for m in range(KO):
                        nc.tensor.matmul(
                            yps[m][:],
                            wo_sb[:, ki, m * P:(m + 1) * P],
                            wo_rhs[:, ki, :],
                            start=(idx == 0), stop=False,
                        )
                for m in range(KO):
                    for idx, ki in enumerate((1, 3, 5, 7)):
                        nc.tensor.matmul(
                            yps[m][:],
                            wo_sb[:, ki, m * P:(m + 1) * P],
                            wo_rhs[:, ki, :],
                            start=False, stop=(idx == 3),
                        )
                    nc.scalar.activation(
                        y_sb[:, m, :], yps[m][:],
                        mybir.ActivationFunctionType.Identity,
                        bias=bo_sb[:, m:m + 1])
                    sq = scratch.tile([P, W5], F32, name="sq_sb", tag="sq")
                    nc.scalar.activation(
                        sq[:], y_sb[:, m, :],
                        mybir.ActivationFunctionType.Square,
                        accum_out=s2_sb[:, m:m + 1])
                    nc.vector.reduce_sum(
                        s1_sb[:, m:m + 1], y_sb[:, m, :],
                        axis=mybir.AxisListType.X)

                # ---- stats AllReduce + BN apply + residual ----
                stats_sb = hpool.tile([P, 2 * KO], F32, name="stats_sb",
                                      tag="stats")
                nc.vector.tensor_copy(stats_sb[:, 0:KO], s1_sb[:])
                nc.vector.tensor_copy(stats_sb[:, KO:2 * KO], s2_sb[:])
                st_in = drampool.tile([P, 2 * KO], F32, name="st_in")
                st_out = drampool.tile([P, 2 * KO], F32, name="st_out")
                nc.sync.dma_start(st_in[:], stats_sb[:])
                nc.gpsimd.collective_compute(
                    "AllReduce",
                    mybir.AluOpType.add,
                    replica_groups=[list(range(NCORES))],
                    ins=[st_in[:].opt()],
                    outs=[st_out[:].opt()],
                )
                gstats_sb = hpool.tile([P, 2 * KO], F32, name="gstats_sb",
                                       tag="gstats")
                nc.sync.dma_start(gstats_sb[:], st_out[:])

                mean_sb = hpool.tile([P, KO], F32, name="mean_sb", tag="mean")
                nc.vector.tensor_scalar_mul(
                    mean_sb[:], gstats_sb[:, 0:KO], 1.0 / NBT)
                var_sb = hpool.tile([P, KO], F32, name="var_sb", tag="var")
                nc.vector.tensor_scalar_mul(
                    var_sb[:], gstats_sb[:, KO:2 * KO], 1.0 / NBT)
                msq_sb = hpool.tile([P, KO], F32, name="msq_sb", tag="msq")
                nc.vector.tensor_tensor(
                    msq_sb[:], mean_sb[:], mean_sb[:], mybir.AluOpType.mult)
                nc.vector.tensor_tensor(
                    var_sb[:], var_sb[:], msq_sb[:], mybir.AluOpType.subtract)
                # rstd = 1/sqrt(var + eps)
                rstd_sb = hpool.tile([P, KO], F32, name="rstd_sb", tag="rstd")
                nc.scalar.activation(
                    rstd_sb[:], var_sb[:],
                    mybir.ActivationFunctionType.Sqrt, bias=eps_sb[:])
                nc.vector.reciprocal(rstd_sb[:], rstd_sb[:])
                # scale = gamma * rstd ; shift = beta - mean * scale
                scl_sb = hpool.tile([P, KO], F32, name="scl_sb", tag="scl")
                nc.vector.tensor_tensor(
                    scl_sb[:], gamma_sb[:], rstd_sb[:], mybir.AluOpType.mult)
                sh_sb = hpool.tile([P, KO], F32, name="sh_sb", tag="sh")
                nc.vector.tensor_tensor(
                    sh_sb[:], mean_sb[:], scl_sb[:], mybir.AluOpType.mult)
                nc.vector.tensor_tensor(
                    sh_sb[:], beta_sb[:], sh_sb[:], mybir.AluOpType.subtract)

                for m in range(KO):
                    nc.scalar.activation(
                        y_sb[:, m, :], y_sb[:, m, :],
                        mybir.ActivationFunctionType.Identity,
                        bias=sh_sb[:, m:m + 1], scale=scl_sb[:, m:m + 1])
                    nc.vector.tensor_tensor(
                        y_sb[:, m, :], y_sb[:, m, :],
                        xres_sb[:, m, :], mybir.AluOpType.add)
                    nc.sync.dma_start(out_d[:, m, :], y_sb[:, m, :])

    nc.compile()
    return nc


def kernel(**inputs) -> np.ndarray:
    global _cached_nc, LAST_RESULT
    x = np.ascontiguousarray(inputs["x"], dtype=np.float32)
    Wq = np.asarray(inputs["Wq"], dtype=np.float32)
    Wk = np.asarray(inputs["Wk"], dtype=np.float32)
    Wv = np.asarray(inputs["Wv"], dtype=np.float32)
    Wo = np.asarray(inputs["Wo"], dtype=np.float32)
    bq = np.asarray(inputs["bq"], dtype=np.float32)
    bk = np.asarray(inputs["bk"], dtype=np.float32)
    bv = np.asarray(inputs["bv"], dtype=np.float32)
    bo = np.asarray(inputs["bo"], dtype=np.float32)
    gamma = np.asarray(inputs["gamma"], dtype=np.float32)
    beta = np.asarray(inputs["beta"], dtype=np.float32)

    if _cached_nc is None:
        _cached_nc = _build()
    nc = _cached_nc

    bf = ml_dtypes.bfloat16

    def pmaj(arr2d, dtype=bf):
        """[C_in, N] -> partition-major [128, C_in//128, N]."""
        ci, n = arr2d.shape
        return np.ascontiguousarray(
            arr2d.reshape(ci // P, P, n).transpose(1, 0, 2)).astype(dtype)

    bo_eff = (bo + Wo @ bv).astype(np.float32)
    woT_pm = pmaj(np.ascontiguousarray(Wo.T))
    wT = {"q": np.ascontiguousarray(Wq.T),
          "k": np.ascontiguousarray(Wk.T),
          "v": np.ascontiguousarray(Wv.T)}
    # x in [NW, 128, KO, 512] quarter/partition-major bf16, per batch
    x_pm = []
    for b in range(B):
        xb = pmaj(x[b])                      # [128, 8, 2048]
        x_pm.append(np.ascontiguousarray(
            xb.reshape(P, KO, NW, W5).transpose(2, 0, 1, 3)))
    sel = np.zeros((P, P), dtype=np.float32)
    sel[0, :DH] = 1.0
    sel[DH, DH:] = 1.0
    sel = sel.astype(bf)

    in_maps = []
    for c in range(NCORES):
        b, g = c // 4, c % 4
        cs = slice(256 * g, 256 * (g + 1))
        xres = np.concatenate(
            [x[0][:, CS * c:CS * (c + 1)],
             x[1][:, CS * c:CS * (c + 1)]], axis=1)   # [C, 512]
        in_maps.append({
            "xkv": x_pm[b],
            "wqT": pmaj(wT["q"][:, cs]),
            "wkT": pmaj(wT["k"][:, cs]),
            "wvT": pmaj(wT["v"][:, cs]),
            "woT": woT_pm,
            "bq": np.ascontiguousarray(bq[cs]),
            "bk": np.ascontiguousarray(bk[cs]),
            "bo": bo_eff, "gamma": gamma, "beta": beta, "sel": sel,
            "xres": pmaj(xres, np.float32),
        })

    res = run_bass_kernel_spmd(
        nc, in_maps, core_ids=list(range(NCORES)), trace=TRACE)
    LAST_RESULT = res

    out = np.empty((B, C, T), dtype=np.float32)
    for c in range(NCORES):
        arr = np.asarray(res.results[c]["out"])   # [128, KO, 512]
        arr = arr.transpose(1, 0, 2).reshape(C, B * CS)
        for b in range(B):
            out[b][:, CS * c:CS * (c + 1)] = arr[:, CS * b:CS * (b + 1)]
    return out
